# revision 5
# baseline (speedup 1.0000x reference)
"""Self-contained Trainium2 Bass kernel for one GPT-2-style transformer
block (B=4, T=2048, C=768, 12 heads, exact-erf GELU MLP), running SPMD on
8 NeuronCores with ZERO cross-core communication.

Sharding: core c = (batch b = c//2, part p = c%2). Each core computes the
full block output for 1024 of its batch's 2048 tokens, chosen as two
512-token q-blocks balanced over the causal triangle (p=0: blocks {0,3},
p=1: blocks {1,2}). The host permutes the 4 token-blocks of x per core so
the program is identical on every core (q-blocks always at block
positions 1 and 3); causal masks arrive as per-core data (iota-threshold
inputs). K/V are computed on-device for all tokens each core's queries
can see. No collectives, no DRAM bounce.

Entry point: kernel(**inputs) -> np.ndarray  (full [4, 2048, 768] output).
"""

import sys

sys.path.insert(0, "/opt/trn_rl_repo")


import concourse.tile as tile
from concourse.vector_clock import ScopedClock, VectorClock


def _patched_drain_and_barrier(self, tick_clock, wait_clock):
    nc = self.nc
    gc = tick_clock.global_clock

    # One NOP per active processor, each carrying at most one sem wait.
    for proc in range(len(gc)):
        tick = gc[proc]
        if tick <= 0:
            continue
        vc = VectorClock()
        vc.require_at_least(proc, tick)
        nop = nc.sync.nop(nofuse=True)
        wait_clock.add_sem_waits(nop.ins, ScopedClock({None: vc}))

    nc.sync.drain()

    nc.all_engine_barrier()
    assert self.sems is not None
    popped = nc._tile_sem_poison_stack.pop()
    assert popped is self._sem_poison
    nc.clear_and_free_semaphores(list(self.sems.allocated().values()))
    nc.all_engine_barrier()


import json

import concourse.bass as bass_mod

_WSPLIT = [0]


def _split_waits_json(bir: bytes) -> bytes:
    """walrus here accepts at most ONE sync wait per instruction; hoist the
    extras onto same-engine NoOps inserted right before the instruction."""
    j = json.loads(bir)
    changed = False
    for f in j.get("functions", []):
        for b in f.get("blocks", []):
            out = []
            for inst in b.get("instructions", []):
                si = inst.get("sync_info")
                waits = (si or {}).get("on_wait") or []
                if len(waits) > 1:
                    changed = True
                    for w in waits[:-1]:
                        _WSPLIT[0] += 1
                        out.append({
                            "debug": inst.get("debug", 0),
                            "engine": inst["engine"],
                            "ins": [],
                            "outs": [],
                            "name": f"I-wsplit-{_WSPLIT[0]}",
                            "opcode": "NoOp",
                            "sync_info": {"on_update": [], "on_wait": [w]},
                        })
                    si["on_wait"] = [waits[-1]]
                out.append(inst)
            b["instructions"] = out
    if not changed:
        return bir
    return json.dumps(j).encode()


_orig_to_json_bytes = bass_mod.Bass.to_json_bytes


def _patched_to_json_bytes(self):
    return _split_waits_json(_orig_to_json_bytes(self))


def apply():
    tile.TileContext._drain_and_barrier = _patched_drain_and_barrier
    bass_mod.Bass.to_json_bytes = _patched_to_json_bytes


apply()


import numpy as np
import concourse.bass as bass
import concourse.tile as tile
from concourse import mybir

F32 = mybir.dt.float32
F32R = mybir.dt.float32r
F16 = mybir.dt.float16
F8 = mybir.dt.float8e4
AF = mybir.ActivationFunctionType
PM = mybir.MatmulPerfMode
OP = mybir.AluOpType

C = 768
CC = 6       # 128-chunks of C
D = 64
H = 12       # all heads on every core
HID = 3072
HC = 24      # 128-chunks of HID
EPS = 1e-5
SCALE = 0.125   # 1/sqrt(64)
EXPB = -4.0     # uniform exp bias; cancels in softmax normalization

QPOS = (1, 3)       # x block-positions holding the two q-slots (uniform)
SLOT_NC = (8, 16)   # key 128-chunks processed per slot


def build(n_cores: int = 8, T: int = 2048, phase_limit: int = 99):
    NB = T // 512       # 4 token blocks
    TQ = T // 2         # 1024 output tokens per core

    nc = bass.Bass("TRN2", target_bir_lowering=False, debug=False,
                   num_devices=n_cores)

    dp = lambda name, shape, dt=F32, out=False: nc.declare_dram_parameter(
        name, shape, dt, isOutput=out)

    xT = dp("xT", [C, T], F16)          # per-core block-permuted, transposed
    wq = dp("wq", [C, C], F16)          # ln1_g folded in
    wk = dp("wk", [C, C], F16)
    wv = dp("wv", [C, C], F16)
    wo = dp("wo", [C, C], F16)
    wfc = dp("wfc", [C, HID], F16)      # ln2_g folded in
    wproj = dp("wproj", [HID, C], F16)
    ebias_in = dp("ebias", [128, 12], F32)      # per-group exp bias (-4/-50)
    outT = dp("outT", [C, TQ], out=True)

    with (
        nc.allow_low_precision(reason="fp16 matmuls"),
        tile.TileContext(nc) as tc,
        tc.tile_pool(name="const", bufs=1) as constp,
        tc.tile_pool(name="tmp2", bufs=3) as tmp2,
        tc.tile_pool(name="obuf", bufs=3) as obuf,
        tc.tile_pool(name="late", bufs=1) as late,      # outlives attn pools
    ):
        # ---------------- constants ----------------
        ones_col16 = constp.tile([128, 1], F16)
        nc.vector.memset(ones_col16, 1.0)
        ones_col_r = constp.tile([128, 1], F32R)
        nc.vector.memset(ones_col_r.bitcast(F32), 1.0)
        ones_row_r = constp.tile([1, 128], F32R)
        nc.vector.memset(ones_row_r.bitcast(F32), 1.0)
        ones_p65 = constp.tile([65, 64], F16)
        nc.vector.memset(ones_p65, 1.0)
        eps_t = constp.tile([1, 1], F32)
        nc.vector.memset(eps_t, EPS)

        # tiles that outlive the attention scope (LIFO pool stacking)
        yT = late.tile([128, CC, TQ], F16)          # attn out, both slots
        x1 = late.tile([128, CC, TQ], F32R)         # residual after attn (rounded for PE stats)
        ln2x = late.tile([128, CC, TQ], F16)

        def layernorm_block(x_chunks, xsq_chunks, emit_z, st, bc, lhsT_ones,
                            statp):
            """LN over one 512-token block. x_chunks/xsq_chunks: lists of CC
            [128,512] APs. emit_z(k, bc) writes the normalized output."""
            for k in range(CC):
                nc.tensor.matmul(st[:, 0, :], lhsT_ones, x_chunks[k],
                                 start=(k == 0), stop=(k == CC - 1))
            for k in range(CC):
                nc.tensor.matmul(st[:, 1, :], lhsT_ones, xsq_chunks[k],
                                 start=(k == 0), stop=(k == CC - 1))
            m = statp.tile([1, 512], F32R, tag="m")
            nc.vector.tensor_scalar_mul(m, st[:, 0, :], 1.0 / C)
            msq = statp.tile([1, 512], F32, tag="msq")
            nc.vector.tensor_mul(msq, m.bitcast(F32), m.bitcast(F32))
            var = statp.tile([1, 512], F32, tag="var")
            # var = st1/C - m^2
            nc.vector.scalar_tensor_tensor(
                out=var, in0=st[:, 1, :], scalar=1.0 / C, in1=msq,
                op0=OP.mult, op1=OP.subtract)
            sd = statp.tile([1, 512], F32, tag="sd")
            nc.scalar.activation(sd, var, AF.Sqrt, bias=eps_t)
            rstd = statp.tile([1, 512], F32R, tag="rstd")
            nc.vector.reciprocal(rstd, sd)
            nc.tensor.matmul(bc[:, 0, :], ones_row_r, m, start=True, stop=True)
            nc.tensor.matmul(bc[:, 1, :], ones_row_r, rstd, start=True, stop=True)
            for k in range(CC):
                emit_z(k, bc)

        # =================================================================
        with (
            tc.tile_pool(name="attA", bufs=1) as attA,
        ):
            xb = attA.tile([128, CC, T], F16)       # permuted x, all blocks
            def emit_x_dmas(i):
                for k in range(CC):
                    nc.sync.dma_start(out=xb[:, k, 512 * i:512 * (i + 1)],
                                      in_=xT[128 * k:128 * (k + 1), 512 * i:512 * (i + 1)])
            emit_x_dmas(0)

            kT = attA.tile([128, CC, T], F16)
            qT = attA.tile([128, CC, TQ], F16)
            v_aug = attA.tile([128, T // 128, H, 65], F16)
            nc.vector.memset(v_aug[:, :, :, 64:65], 1.0)

            with (
                tc.tile_pool(name="wqkv", bufs=1) as wqkvp,
                tc.tile_pool(name="lnbuf", bufs=1) as lnbuf,
                tc.tile_pool(name="x2p", bufs=1) as x2p,
                tc.tile_pool(name="statp", bufs=1) as statp,
                tc.tile_pool(name="ps_st", bufs=1, space="PSUM") as ps_st,
                tc.tile_pool(name="ps_bc", bufs=1, space="PSUM") as ps_bc,
                tc.tile_pool(name="ps_k", bufs=2, space="PSUM") as ps_k,
                tc.tile_pool(name="ps_v", bufs=2, space="PSUM") as ps_v,
            ):
                wq_sb = wqkvp.tile([128, CC, C], F16)
                wk_sb = wqkvp.tile([128, CC, C], F16)
                wv_sb = wqkvp.tile([128, CC, C], F16)
                for k in range(CC):
                    nc.sync.dma_start(out=wk_sb[:, k, :], in_=wk[128 * k:128 * (k + 1), :])
                emit_x_dmas(1)
                for k in range(CC):
                    nc.sync.dma_start(out=wv_sb[:, k, :], in_=wv[128 * k:128 * (k + 1), :])
                emit_x_dmas(2)
                for k in range(CC):
                    nc.sync.dma_start(out=wq_sb[:, k, :], in_=wq[128 * k:128 * (k + 1), :])
                emit_x_dmas(3)
                lnq = wqkvp.tile([128, CC, TQ], F16)    # ln1 of q-blocks

                # ---- per block: LN1 -> K,V projections ----
                for i in range(NB):
                    qi = QPOS.index(i) if i in QPOS else None
                    if qi is None:
                        dst = lnbuf.tile([128, CC, 512], F16, tag="lnblk")
                        dsl = lambda k, _d=dst: _d[:, k, :]
                    else:
                        dsl = lambda k, _q=qi: lnq[:, k, 512 * _q:512 * (_q + 1)]
                    xsl = lambda k, _i=i: xb[:, k, 512 * _i:512 * (_i + 1)]
                    x2 = x2p.tile([128, CC, 512], F16, tag="x2")
                    for k in range(CC):
                        nc.gpsimd.tensor_mul(x2[:, k, :], xsl(k), xsl(k))
                    st = ps_st.tile([1, 2, 512], F32, tag="st")
                    bc = ps_bc.tile([128, 2, 512], F32, tag="bc")

                    def emit_z(k, bc_, _dsl=dsl, _xsl=xsl):
                        t1 = tmp2.tile([128, 512], F32, tag="t1")
                        nc.vector.tensor_sub(t1, _xsl(k), bc_[:, 0, :])
                        nc.vector.tensor_mul(_dsl(k), t1, bc_[:, 1, :])

                    layernorm_block([xsl(k) for k in range(CC)],
                                    [x2[:, k, :] for k in range(CC)],
                                    emit_z, st, bc, ones_col16, statp)

                    # K projection for this block
                    for co in range(CC):
                        pk = ps_k.tile([128, 512], F32, tag="pk")
                        for ci in range(CC):
                            nc.tensor.matmul(pk, wk_sb[:, ci, 128 * co:128 * (co + 1)],
                                             dsl(ci), start=(ci == 0), stop=(ci == CC - 1))
                        nc.any.tensor_copy(kT[:, co, 512 * i:512 * (i + 1)], pk)
                    # V projection for this block (tokens on partitions)
                    for t in range(4):
                        for g in range(2):
                            pv = ps_v.tile([128, 384], F32, tag="pv")
                            for ci in range(CC):
                                nc.tensor.matmul(
                                    pv, dsl(ci)[:, 128 * t:128 * (t + 1)],
                                    wv_sb[:, ci, 384 * g:384 * (g + 1)],
                                    start=(ci == 0), stop=(ci == CC - 1))
                            nc.any.tensor_copy(
                                v_aug[:, 4 * i + t, 6 * g:6 * (g + 1), 0:64],
                                pv.rearrange("p (h d) -> p h d", h=6))

                # ---- Q projection (both slots) ----
                for s in range(2):
                    for co in range(CC):
                        pq = ps_k.tile([128, 512], F32, tag="pk")
                        for ci in range(CC):
                            nc.tensor.matmul(pq, wq_sb[:, ci, 128 * co:128 * (co + 1)],
                                             lnq[:, ci, 512 * s:512 * (s + 1)],
                                             start=(ci == 0), stop=(ci == CC - 1))
                        nc.any.tensor_copy(qT[:, co, 512 * s:512 * (s + 1)], pq)

            def dumpq(chunks):
                for k in range(CC):
                    ob = obuf.tile([128, TQ], F32, tag="dump")
                    nc.vector.tensor_copy(ob, chunks[k])
                    nc.sync.dma_start(out=outT[128 * k:128 * (k + 1), :], in_=ob)

            if phase_limit <= 1:
                dumpq([kT[:, k, 0:TQ] for k in range(CC)])
                return nc
            if phase_limit <= 2:
                dumpq([qT[:, k, :] for k in range(CC)])
                return nc

            # ---- attention: pipelined scores/exp/mask/AV + delayed norm ----
            with tc.tile_pool(name="attB", bufs=1) as attB:
                # static diagonal mask: tri[i, col] = 1 iff i <= col - 512
                tri = attB.tile([128, 1024], F16)
                nc.gpsimd.memset(tri, 1.0)
                nc.gpsimd.affine_select(
                    out=tri, in_=tri, compare_op=OP.is_ge, fill=0.0,
                    base=-512, pattern=[[1, 1024]], channel_multiplier=-1)
                ebias_sb = attB.tile([128, 12], F32)
                nc.sync.dma_start(out=ebias_sb, in_=ebias_in[:, :])
                # wo load (DMA early, used at O-proj)
                wo_sb = attB.tile([128, CC, C], F16)
                for k in range(CC):
                    nc.sync.dma_start(out=wo_sb[:, k, :], in_=wo[128 * k:128 * (k + 1), :])
                yraw = attB.tile([65, 24, 512], F16)    # unnormalized y + 1/den row

                attloop = tc.tile_pool(name="attwork", bufs=1)
                attw = attloop.__enter__()
                expp_cm = tc.tile_pool(name="expp", bufs=4)
                expp = expp_cm.__enter__()
                ynp_cm = tc.tile_pool(name="ynorm", bufs=4)
                ynp = ynp_cm.__enter__()
                ps_s_cm = tc.tile_pool(name="ps_s", bufs=3, space="PSUM")
                ps_s = ps_s_cm.__enter__()
                ps_y_cm = tc.tile_pool(name="ps_y", bufs=1, space="PSUM")
                ps_y = ps_y_cm.__enter__()
                ps_n_cm = tc.tile_pool(name="ps_n", bufs=1, space="PSUM")
                ps_n = ps_n_cm.__enter__()

                flat = []
                for s in range(2):
                    NG = SLOT_NC[s] // 2
                    for h in range(H):
                        for g in range(NG):
                            flat.append((s, h, g, NG))
                psm_t = [None, None, None]
                ex_t = [None, None, None]
                py_t = {}

                def emit_scores(idx):
                    s, h, g, NG = flat[idx]
                    hp = 64 * (h % 2)
                    kch = h // 2
                    psm = ps_s.tile([128, 2, 512], F32, tag="psm")
                    for j in range(2):
                        kc = 2 * g + j
                        nc.tensor.matmul(
                            psm[:, j, :],
                            kT[hp:hp + 64, kch, 128 * kc:128 * (kc + 1)],
                            qT[hp:hp + 64, kch, 512 * s:512 * (s + 1)],
                            start=True, stop=True)
                    psm_t[idx % 3] = psm

                def emit_expmask(idx):
                    s, h, g, NG = flat[idx]
                    psm = psm_t[idx % 3]
                    gb = g if s == 0 else 4 + g     # ebias column
                    ex = expp.tile([128, 2, 512], F16, tag="ex")
                    nc.scalar.activation(
                        ex.rearrange("p a b -> p (a b)"),
                        psm.rearrange("p a b -> p (a b)"),
                        AF.Exp, scale=SCALE, bias=ebias_sb[:, gb:gb + 1])
                    # diagonal groups: static triangle masks (q-block is always
                    # at key-positions 1 (slot X) and 3 (slot Y))
                    diag = (s == 0 and g >= 2) or (s == 1 and g >= 6)
                    if diag:
                        s0 = 2 * (g - (2 if s == 0 else 6))
                        for j in range(2):
                            sj = s0 + j
                            nc.vector.tensor_mul(
                                ex[:, j, :], ex[:, j, :],
                                tri[:, 512 - 128 * sj:1024 - 128 * sj])
                    ex_t[idx % 3] = ex

                def emit_av(idx):
                    s, h, g, NG = flat[idx]
                    ex = ex_t[idx % 3]
                    if g == 0:
                        py_t[(s, h)] = ps_y.tile([65, 512], F32, tag="py", name="py")
                    py = py_t[(s, h)]
                    for j in range(2):
                        kc = 2 * g + j
                        nc.tensor.matmul(py, v_aug[:, kc, h, :], ex[:, j, :],
                                         start=(kc == 0), stop=(kc == 2 * NG - 1))

                from collections import deque
                pend = deque()

                def emit_finish(s, h):
                    row = 12 * s + h
                    hp = 64 * (h % 2)
                    kch = h // 2
                    pbc = ps_n.tile([64, 512], F32, tag="pbc", name="pbc")
                    nc.tensor.matmul(pbc, ones_p65[64:65, :],
                                     yraw[64:65, row, :], start=True, stop=True)
                    yn = ynp.tile([64, 512], F16, tag="yn")
                    nc.vector.tensor_mul(yn, yraw[0:64, row, :], pbc)
                    nc.sync.dma_start(
                        out=yT[hp:hp + 64, kch, 512 * s:512 * (s + 1)], in_=yn)

                emit_scores(0)
                emit_scores(1)
                for idx in range(len(flat)):
                    if idx + 2 < len(flat):
                        emit_scores(idx + 2)
                    emit_expmask(idx)
                    emit_av(idx)
                    s, h, g, NG = flat[idx]
                    if g == NG - 1:
                        row = 12 * s + h
                        py = py_t.pop((s, h))
                        nc.vector.tensor_copy(yraw[:, row, :], py)
                        nc.vector.reciprocal(yraw[64:65, row, :],
                                             yraw[64:65, row, :])
                        pend.append((s, h, idx + 2))
                    while pend and pend[0][2] <= idx:
                        ss, hh, _ = pend.popleft()
                        emit_finish(ss, hh)
                while pend:
                    ss, hh, _ = pend.popleft()
                    emit_finish(ss, hh)

                ps_n_cm.__exit__(None, None, None)
                ps_y_cm.__exit__(None, None, None)
                ps_s_cm.__exit__(None, None, None)
                ynp_cm.__exit__(None, None, None)
                expp_cm.__exit__(None, None, None)
                attloop.__exit__(None, None, None)

                # ---- normalize + O-projection + residual + LN2 ----
                with (
                    tc.tile_pool(name="x2b", bufs=1) as x2b,
                    tc.tile_pool(name="stat2", bufs=1) as stat2,
                    tc.tile_pool(name="ps_po", bufs=3, space="PSUM") as ps_po,
                    tc.tile_pool(name="ps_st2", bufs=1, space="PSUM") as ps_st2,
                    tc.tile_pool(name="ps_bc2", bufs=1, space="PSUM") as ps_bc2,
                ):
                    def emit_oproj(s):
                        xoff = 512 * QPOS[s]
                        for co in range(CC):
                            po = ps_po.tile([128, 512], F32, tag="po")
                            for ci in range(CC):
                                nc.tensor.matmul(po, wo_sb[:, ci, 128 * co:128 * (co + 1)],
                                                 yT[:, ci, 512 * s:512 * (s + 1)],
                                                 start=(ci == 0), stop=(ci == CC - 1))
                            nc.vector.tensor_add(
                                x1[:, co, 512 * s:512 * (s + 1)], po,
                                xb[:, co, xoff:xoff + 512])

                    def emit_ln2(s):
                        xsl = lambda k, _s=s: x1[:, k, 512 * _s:512 * (_s + 1)]
                        x2 = x2b.tile([128, CC, 512], F32R, tag="x2")
                        for k in range(CC):
                            nc.gpsimd.tensor_mul(x2[:, k, :], xsl(k).bitcast(F32),
                                                 xsl(k).bitcast(F32))
                        st = ps_st2.tile([1, 2, 512], F32, tag="st")
                        bc = ps_bc2.tile([128, 2, 512], F32, tag="bc")

                        def emit_z2(k, bc_, _s=s):
                            t1 = tmp2.tile([128, 512], F32, tag="t1")
                            nc.vector.tensor_sub(
                                t1, x1.bitcast(F32)[:, k, 512 * _s:512 * (_s + 1)],
                                bc_[:, 0, :])
                            nc.vector.tensor_mul(
                                ln2x[:, k, 512 * _s:512 * (_s + 1)], t1, bc_[:, 1, :])

                        layernorm_block(
                            [xsl(k) for k in range(CC)],
                            [x2[:, k, :] for k in range(CC)],
                            emit_z2, st, bc, ones_col_r, stat2)

                    emit_oproj(0)
                    emit_ln2(0)
                    emit_oproj(1)
                    emit_ln2(1)

                if phase_limit <= 3:
                    dumpq([yT[:, k, :] for k in range(CC)])
                    return nc

        if phase_limit <= 4:
            for k in range(CC):
                ob = obuf.tile([128, TQ], F32, tag="dump")
                nc.vector.tensor_copy(ob, x1.bitcast(F32)[:, k, :])
                nc.sync.dma_start(out=outT[128 * k:128 * (k + 1), :], in_=ob)
            return nc

        # ================= MLP =================
        if phase_limit <= 5:
            for k in range(CC):
                ob = obuf.tile([128, TQ], F32, tag="dump")
                nc.vector.tensor_copy(ob, ln2x[:, k, :])
                nc.sync.dma_start(out=outT[128 * k:128 * (k + 1), :], in_=ob)
            return nc

        with (
            tc.tile_pool(name="mlpA", bufs=1) as mlpA,
            tc.tile_pool(name="ps_mlp", bufs=3, space="PSUM") as ps_mlp,
        ):
            wfc_sb = mlpA.tile([128, CC, HID], F16)
            for ci in range(CC):
                nc.sync.dma_start(out=wfc_sb[:, ci, :],
                                  in_=wfc[128 * ci:128 * (ci + 1), :])
            wp_sb = mlpA.tile([128, HC, C], F16)
            for hc in range(HC):
                nc.sync.dma_start(out=wp_sb[:, hc, :],
                                  in_=wproj[128 * hc:128 * (hc + 1), :])

            hg = mlpA.tile([128, HC, TQ], F16)
            for q2 in range(2):
                for hc in range(HC):
                    ph = ps_mlp.tile([128, 512], F32, tag="ph")
                    for ci in range(CC):
                        nc.tensor.matmul(
                            ph, wfc_sb[:, ci, 128 * hc:128 * (hc + 1)],
                            ln2x[:, ci, 512 * q2:512 * (q2 + 1)],
                            start=(ci == 0), stop=(ci == CC - 1))
                    nc.scalar.activation(hg[:, hc, 512 * q2:512 * (q2 + 1)],
                                         ph, AF.Gelu)

            if phase_limit <= 6:
                for k in range(CC):
                    ob = obuf.tile([128, TQ], F32, tag="dump")
                    nc.vector.tensor_copy(ob, hg[:, k, :])
                    nc.sync.dma_start(out=outT[128 * k:128 * (k + 1), :], in_=ob)
                return nc

            for q2 in range(2):
                for co in range(CC):
                    pm = ps_mlp.tile([128, 512], F32, tag="pm")
                    for hc in range(HC):
                        nc.tensor.matmul(
                            pm, wp_sb[:, hc, 128 * co:128 * (co + 1)],
                            hg[:, hc, 512 * q2:512 * (q2 + 1)],
                            start=(hc == 0), stop=(hc == HC - 1))
                    ob = obuf.tile([128, 512], F32, tag="ob")
                    nc.vector.tensor_add(
                        ob, pm, x1.bitcast(F32)[:, co, 512 * q2:512 * (q2 + 1)])
                    nc.sync.dma_start(
                        out=outT[128 * co:128 * (co + 1), 512 * q2:512 * (q2 + 1)],
                        in_=ob)

    return nc


# ---------------- host-side sharding ----------------

# block permutation per part p (position -> original block index)
PERM = ((1, 0, 2, 3), (0, 1, 3, 2))


def _exp_biases(p):
    """12 per-group exp biases: normal -4, fully-masked groups -50.
    Groups: slot X g0..3 (chunks 0..7), slot Y g0..7 (chunks 0..15)."""
    e = [-4.0] * 12
    if p == 0:
        e[0] = e[1] = -50.0      # X pos0 = block1 (future)
    else:
        e[8] = e[9] = -50.0      # Y pos2 = block3 (future)
    return np.array(e, dtype=np.float32)


def shard_inputs(inputs, n_cores=8):
    x = np.asarray(inputs["x"])
    W_attn = np.asarray(inputs["W_attn"], dtype=np.float32)
    W_o = np.asarray(inputs["W_o"], dtype=np.float32)
    W_fc = np.asarray(inputs["W_fc"], dtype=np.float32)
    W_proj = np.asarray(inputs["W_proj"], dtype=np.float32)
    g1 = np.asarray(inputs["ln1_g"], dtype=np.float32)
    b1 = np.asarray(inputs["ln1_b"], dtype=np.float32)
    g2 = np.asarray(inputs["ln2_g"], dtype=np.float32)
    b2 = np.asarray(inputs["ln2_b"], dtype=np.float32)
    assert np.all(b1 == 0.0) and np.all(b2 == 0.0), "ln biases must be zero"
    B, T, _ = x.shape
    f16 = lambda a: np.ascontiguousarray(a, dtype=np.float16)

    wq_h = f16(g1[:, None] * W_attn[:, 0:C])
    wk_h = f16(g1[:, None] * W_attn[:, C:2 * C])
    wv_h = f16(g1[:, None] * W_attn[:, 2 * C:3 * C])
    wo_h = f16(W_o)
    wfc_h = f16(g2[:, None] * W_fc)
    wproj_h = f16(W_proj)

    in_maps = []
    for c in range(n_cores):
        b, p = c // 2, c % 2
        xp = x[b].reshape(4, 512, C)[list(PERM[p])]     # [4, 512, C]
        xTp = f16(xp.reshape(T, C).T)                   # [C, T] permuted
        eb = np.ascontiguousarray(
            np.broadcast_to(_exp_biases(p)[None, :], (128, 12)).astype(np.float32))
        in_maps.append({
            "xT": xTp,
            "wq": wq_h, "wk": wk_h, "wv": wv_h, "wo": wo_h,
            "wfc": wfc_h, "wproj": wproj_h,
            "ebias": eb,
        })
    return in_maps


def unshard(results, n_cores=8, T=2048):
    out = np.empty((n_cores // 2, T, C), np.float32)
    for c in range(n_cores):
        b, p = c // 2, c % 2
        o = results[c]["outT"]                  # [C, 1024]
        qa, qb = (0, 3) if p == 0 else (1, 2)
        out[b, 512 * qa:512 * (qa + 1)] = o[:, 0:512].T
        out[b, 512 * qb:512 * (qb + 1)] = o[:, 512:1024].T
    return out


_CACHED = {}


def kernel(**inputs):
    import numpy as np
    from concourse.bass_utils import run_bass_kernel_spmd

    n_cores, T = 8, 2048
    if "nc" not in _CACHED:
        _CACHED["nc"] = build(n_cores=n_cores, T=T)
    nc = _CACHED["nc"]
    in_maps = shard_inputs(inputs, n_cores=n_cores)
    res = run_bass_kernel_spmd(nc, in_maps, core_ids=list(range(n_cores)))
    return unshard(res.results, n_cores=n_cores, T=T)
